# revision 13
# baseline (speedup 1.0000x reference)
"""Trainium2 Bass kernel for nn_BernMLPAugmenter (gnn_message_passing).

Computes, for each of 400k edges:
    edge_emb = concat(node_emb[src], node_emb[dst])        # [2D]
    logit    = W2.T @ relu(W1.T @ edge_emb + b1) + b2      # scalar
    eps      = (2*BIAS-1)*eps_u + (1-BIAS)
    aug_w    = sigmoid(log(eps) - log1p(-eps) + logit)
    new_vals = vals * aug_w
Returns (concat([new_vals, new_vals]), mean(aug_w)).

Sharding: edges data-parallel across 8 NeuronCores; node_emb + MLP weights
replicated. The scalar mean is reduced on host from per-core partials.

Device pipeline per core (50176 padded edges = 128 partitions x 392):
  - indirect-DMA gather of node_emb rows (512B each) for src/dst, 49-tile
    chunks, double buffered
  - per 128-edge tile: PE transpose of the gathered [e,d] tile to [d,e],
    DVE copy PSUM->SBUF, two accumulating fp32 matmuls against W1 halves
    giving H.T = [128e, 64h] in PSUM, ACT relu, DVE fused mul+reduce with
    replicated W2 -> per-edge logits
  - one bulk elementwise gate phase (ACT Ln/Sigmoid + DVE) over [128, 392]
"""

import os

import numpy as np

import concourse.bass as bass
import concourse.mybir as mybir
import concourse.tile as tile
from concourse.bass import IndirectOffsetOnAxis
from concourse.bass_utils import run_bass_kernel_spmd

# problem constants
N_NODES = 100000
D = 128
EH = 400000
H = 64
BIAS = 1e-4
N_CORES = 8

P = 128

# fp32 constants matching the reference's arithmetic
C0 = np.float32(BIAS - (1.0 - BIAS))          # -0.9998
C1 = np.float32(1.0 - BIAS)                   # 0.9999
C2 = np.float32(-C0)                          # 0.9998  (scale for 1-eps)
C3 = np.float32(1.0) - C1                     # 1 - fl(0.9999), exact f32

f32 = mybir.dt.float32
i32 = mybir.dt.int32

# results of the last device run (for test harness introspection)
LAST_RUN = {}


def split_waits(nc, maxw=1):
    """Walrus codegen rejects >1 sem wait on one instruction; hoist excess
    waits onto chained wait-only drains inserted just before it."""
    n_split = 0
    for f in nc.m.functions:
        for bb in f.blocks:
            idx = 0
            while idx < len(bb.instructions):
                inst = bb.instructions[idx]
                si = inst.sync_info
                if si is not None and si.on_wait is not None and len(si.on_wait) > maxw:
                    waits = list(si.on_wait)
                    keep = waits[-maxw:]
                    extra = waits[:-maxw]
                    pos = idx
                    for k in range(0, len(extra), maxw):
                        chunk = extra[k : k + maxw]
                        d = mybir.InstDrain(
                            name=f"{inst.name}_w{k}",
                            ins=[],
                            outs=[],
                            bass_is_fusable=False,
                        )
                        d.engine = inst.engine
                        d.sync_info = mybir.SyncInfo(on_wait=chunk, on_update=[])
                        nc.register_instruction(d, overwrite=True)
                        bb.instructions.insert(pos, d)
                        pos += 1
                        idx += 1
                    inst.sync_info = mybir.SyncInfo(
                        on_wait=keep, on_update=list(si.on_update or [])
                    )
                    n_split += 1
                idx += 1
    return n_split


def build_program(n_nodes, ept, t_ch, b1_nonzero):
    """Build the per-core SPMD Bass program.

    ept: edge columns per partition (edges per core = 128*ept)
    t_ch: 128-edge tiles per gather chunk (must divide ept)
    """
    assert ept % t_ch == 0
    n_chunk = ept // t_ch

    nc = bass.Bass()

    emb = nc.declare_dram_parameter("emb", [n_nodes, D], f32, isOutput=False)
    idx = nc.declare_dram_parameter("idx", [P, 2 * ept], i32, isOutput=False)
    vals = nc.declare_dram_parameter("vals", [P, ept], f32, isOutput=False)
    eps = nc.declare_dram_parameter("eps", [P, ept], f32, isOutput=False)
    mask = nc.declare_dram_parameter("mask", [P, ept], f32, isOutput=False)
    w1 = nc.declare_dram_parameter("w1", [2 * D, H], f32, isOutput=False)
    w2r = nc.declare_dram_parameter("w2r", [P, H], f32, isOutput=False)
    b1r = nc.declare_dram_parameter("b1r", [P, H], f32, isOutput=False)
    b2r = nc.declare_dram_parameter("b2r", [P, 1], f32, isOutput=False)
    # bias columns for ACT ops: [C1, C3, 0.0]
    cvec = nc.declare_dram_parameter("cvec", [P, 3], f32, isOutput=False)
    idn = nc.declare_dram_parameter("idn", [P, P], f32, isOutput=False)
    out_nv = nc.declare_dram_parameter("out_nv", [P, ept], f32, isOutput=True)
    out_sum = nc.declare_dram_parameter("out_sum", [P, 1], f32, isOutput=True)

    Relu = mybir.ActivationFunctionType.Relu
    Ln = mybir.ActivationFunctionType.Ln
    Sigmoid = mybir.ActivationFunctionType.Sigmoid

    with tile.TileContext(nc) as tc:
        with (
            tc.tile_pool(name="consts", bufs=1) as cpool,
            tc.tile_pool(name="gather", bufs=8) as gpool,
            tc.tile_pool(name="xsb", bufs=4) as xpool,
            tc.tile_pool(name="hsb", bufs=4) as hpool,
            tc.tile_pool(name="acc", bufs=1) as apool,
            tc.tile_pool(name="ps_t", bufs=4, space="PSUM") as pspool,
            tc.tile_pool(name="ps_h", bufs=4, space="PSUM") as pshpool,
        ):
            # ---- load constants / per-core arrays into SBUF ----
            idx_sb = cpool.tile([P, 2 * ept], i32)
            nc.sync.dma_start(out=idx_sb[:], in_=idx[:])
            vals_sb = cpool.tile([P, ept], f32)
            nc.sync.dma_start(out=vals_sb[:], in_=vals[:])
            eps_sb = cpool.tile([P, ept], f32)
            nc.sync.dma_start(out=eps_sb[:], in_=eps[:])
            mask_sb = cpool.tile([P, ept], f32)
            nc.sync.dma_start(out=mask_sb[:], in_=mask[:])
            w1s_sb = cpool.tile([P, H], f32)
            nc.sync.dma_start(out=w1s_sb[:], in_=w1[0:D, :])
            w1d_sb = cpool.tile([P, H], f32)
            nc.sync.dma_start(out=w1d_sb[:], in_=w1[D : 2 * D, :])
            w2r_sb = cpool.tile([P, H], f32)
            nc.sync.dma_start(out=w2r_sb[:], in_=w2r[:])
            b2r_sb = cpool.tile([P, 1], f32)
            nc.sync.dma_start(out=b2r_sb[:], in_=b2r[:])
            cvec_sb = cpool.tile([P, 3], f32)
            nc.sync.dma_start(out=cvec_sb[:], in_=cvec[:])
            idn_sb = cpool.tile([P, P], f32)
            nc.sync.dma_start(out=idn_sb[:], in_=idn[:])
            if b1_nonzero:
                b1r_sb = cpool.tile([P, H], f32)
                nc.sync.dma_start(out=b1r_sb[:], in_=b1r[:])

            logits = apool.tile([P, ept], f32)

            # ---- main loop: per-tile gather (128 rows per indirect DMA,
            # one index per partition: the only HW-supported form) + MLP ----
            for e_col in range(ept):
                gs = gpool.tile([P, D], f32, tag="gs")
                nc.gpsimd.indirect_dma_start(
                    out=gs[:],
                    out_offset=None,
                    in_=emb[:],
                    in_offset=IndirectOffsetOnAxis(
                        ap=idx_sb[:, e_col : e_col + 1], axis=0
                    ),
                )
                gd = gpool.tile([P, D], f32, tag="gd")
                nc.gpsimd.indirect_dma_start(
                    out=gd[:],
                    out_offset=None,
                    in_=emb[:],
                    in_offset=IndirectOffsetOnAxis(
                        ap=idx_sb[:, ept + e_col : ept + e_col + 1], axis=0
                    ),
                )
                # transpose gathered [128e, 128d] -> [128d, 128e]
                ps_s = pspool.tile([P, P], f32, tag="pt")
                nc.tensor.transpose(out=ps_s[:], in_=gs[:], identity=idn_sb[:])
                xs = xpool.tile([P, P], f32, tag="x")
                nc.vector.tensor_copy(out=xs[:], in_=ps_s[:])
                ps_d = pspool.tile([P, P], f32, tag="pt")
                nc.tensor.transpose(out=ps_d[:], in_=gd[:], identity=idn_sb[:])
                xd = xpool.tile([P, P], f32, tag="x")
                nc.vector.tensor_copy(out=xd[:], in_=ps_d[:])

                # H.T = [128e, 64h] = Xs.T@W1s + Xd.T@W1d
                hp = pshpool.tile([P, H], f32, tag="h")
                nc.tensor.matmul(
                    out=hp[:], lhsT=xs[:], rhs=w1s_sb[:], start=True, stop=False
                )
                nc.tensor.matmul(
                    out=hp[:], lhsT=xd[:], rhs=w1d_sb[:], start=False, stop=True
                )

                hs = hpool.tile([P, H], f32, tag="hs")
                if b1_nonzero:
                    nc.vector.tensor_add(out=hs[:], in0=hp[:], in1=b1r_sb[:])
                    nc.scalar.activation(
                        out=hs[:], in_=hs[:], func=Relu, bias=cvec_sb[:, 2:3]
                    )
                else:
                    nc.scalar.activation(
                        out=hs[:], in_=hp[:], func=Relu, bias=cvec_sb[:, 2:3]
                    )

                # logit[e] = sum_h hs[e,h] * W2[h]   (b2 folded into the
                # sigmoid bias in the gate phase)
                scr = hpool.tile([P, H], f32, tag="scr")
                nc.vector.tensor_mul(out=scr[:], in0=hs[:], in1=w2r_sb[:])
                nc.vector.tensor_reduce(
                    out=logits[:, e_col : e_col + 1],
                    in_=scr[:],
                    axis=mybir.AxisListType.X,
                    op=mybir.AluOpType.add,
                )

            # ---- bulk gate phase over [128, ept] ----
            lg1 = apool.tile([P, ept], f32)
            nc.scalar.activation(
                out=lg1[:], in_=eps_sb[:], func=Ln, scale=float(C0), bias=cvec_sb[:, 0:1]
            )
            lg2 = apool.tile([P, ept], f32)
            nc.scalar.activation(
                out=lg2[:], in_=eps_sb[:], func=Ln, scale=float(C2), bias=cvec_sb[:, 1:2]
            )
            gin = apool.tile([P, ept], f32)
            nc.vector.tensor_sub(out=gin[:], in0=lg1[:], in1=lg2[:])
            gin2 = apool.tile([P, ept], f32)
            nc.vector.tensor_add(out=gin2[:], in0=gin[:], in1=logits[:])
            aug = apool.tile([P, ept], f32)
            nc.scalar.activation(out=aug[:], in_=gin2[:], func=Sigmoid, bias=b2r_sb[:, 0:1])
            nv = apool.tile([P, ept], f32)
            nc.vector.tensor_mul(out=nv[:], in0=vals_sb[:], in1=aug[:])
            am = apool.tile([P, ept], f32)
            nc.vector.tensor_mul(out=am[:], in0=aug[:], in1=mask_sb[:])
            s_sb = apool.tile([P, 1], f32)
            nc.vector.tensor_reduce(
                out=s_sb[:], in_=am[:], axis=mybir.AxisListType.X,
                op=mybir.AluOpType.add,
            )

            nc.sync.dma_start(out=out_nv[:], in_=nv[:])
            nc.sync.dma_start(out=out_sum[:], in_=s_sb[:])

    split_waits(nc)
    return nc


_PROGRAM_CACHE = {}


def _get_program(n_nodes, ept, t_ch, b1_nonzero):
    key = (n_nodes, ept, t_ch, b1_nonzero)
    if key not in _PROGRAM_CACHE:
        _PROGRAM_CACHE[key] = build_program(n_nodes, ept, t_ch, b1_nonzero)
    return _PROGRAM_CACHE[key]


def make_in_maps(node_emb, src, dst, vals, eps_u, W1, b1, W2, b2, n_cores, ept):
    """Shard + pad host inputs into per-core input maps."""
    e_core = src.shape[0] // n_cores
    e_pad = P * ept
    assert e_pad >= e_core

    node_emb = np.ascontiguousarray(np.asarray(node_emb, dtype=np.float32))
    src = np.asarray(src).astype(np.int32)
    dst = np.asarray(dst).astype(np.int32)
    vals = np.asarray(vals, dtype=np.float32).reshape(-1)
    eps = np.asarray(eps_u, dtype=np.float32).reshape(-1)
    W1 = np.ascontiguousarray(np.asarray(W1, dtype=np.float32))
    b1 = np.asarray(b1, dtype=np.float32).reshape(-1)
    W2 = np.asarray(W2, dtype=np.float32).reshape(-1)
    b2 = np.asarray(b2, dtype=np.float32).reshape(-1)

    w2r = np.ascontiguousarray(np.tile(W2[None, :], (P, 1)))
    b1r = np.ascontiguousarray(np.tile(b1[None, :], (P, 1)))
    b2r = np.full((P, 1), b2[0], dtype=np.float32)
    cvec = np.tile(np.array([[C1, C3, 0.0]], dtype=np.float32), (P, 1))
    idn = np.eye(P, dtype=np.float32)
    mask = (np.arange(e_pad) < e_core).astype(np.float32).reshape(P, ept)

    def pad_to(x, fill):
        out = np.full(e_pad, fill, dtype=x.dtype)
        out[:e_core] = x
        return out.reshape(P, ept)

    in_maps = []
    for c in range(n_cores):
        sl = slice(c * e_core, (c + 1) * e_core)
        idx_cat = np.concatenate(
            [pad_to(src[sl], 0), pad_to(dst[sl], 0)], axis=1
        ).astype(np.int32)
        in_maps.append(
            {
                "emb": node_emb,
                "idx": np.ascontiguousarray(idx_cat),
                "vals": np.ascontiguousarray(pad_to(vals[sl], 0.0)),
                "eps": np.ascontiguousarray(pad_to(eps[sl], 0.5)),
                "mask": mask,
                "w1": W1,
                "w2r": w2r,
                "b1r": b1r,
                "b2r": b2r,
                "cvec": cvec,
                "idn": idn,
            }
        )
    return in_maps


def postprocess(results, n_cores, e_core):
    """Assemble full outputs from per-core results."""
    new_vals = np.concatenate(
        [np.asarray(results[c]["out_nv"]).reshape(-1)[:e_core] for c in range(n_cores)]
    )
    total = float(
        np.sum([np.asarray(results[c]["out_sum"], dtype=np.float64).sum()
                for c in range(n_cores)])
    )
    mean = np.float32(total / EH)
    sym_vals = np.concatenate([new_vals, new_vals])
    return sym_vals, mean


def _ensure_ntff_hook():
    """The agent image's antenv lacks axon_hooks; inject it and register the
    ctypes NTFF profile hook so run_bass_kernel_spmd(trace=True) works."""
    import sys
    import types

    try:
        import antenv.axon_hooks  # noqa: F401
        return
    except ImportError:
        pass
    m = types.ModuleType("antenv.axon_hooks")
    state = {"hook": None}
    m.get_axon_ntff_profile_hook = lambda: state["hook"]
    m.set_axon_ntff_profile_hook = lambda h: state.update(hook=h)
    sys.modules["antenv.axon_hooks"] = m
    try:
        from trn_agent_boot.trn_boot import _ntff_profile_via_ctypes

        state["hook"] = _ntff_profile_via_ctypes("/opt/axon/libaxon_pjrt.so")
    except Exception:
        pass


def kernel(node_emb, src, dst, vals, eps_u, W1, b1, W2, b2):
    ept, t_ch = 392, 49  # 128*392 = 50176 padded edges/core, 8 gather chunks
    b1_nonzero = bool(np.any(np.asarray(b1)))
    nc = _get_program(N_NODES, ept, t_ch, b1_nonzero)
    in_maps = make_in_maps(
        node_emb, src, dst, vals, eps_u, W1, b1, W2, b2, N_CORES, ept
    )
    trace = bool(int(os.environ.get("BASS_KERNEL_TRACE", "0")))
    if trace:
        _ensure_ntff_hook()
    res = run_bass_kernel_spmd(nc, in_maps, list(range(N_CORES)), trace=trace)
    LAST_RUN["exec_time_ns"] = res.exec_time_ns
    LAST_RUN["profile_json"] = getattr(res, "profile_json", None)
    return postprocess(res.results, N_CORES, EH // N_CORES)


# revision 14
# speedup vs baseline: 2.7472x; 2.7472x over previous
"""Trainium2 Bass kernel for nn_BernMLPAugmenter (gnn_message_passing).

Computes, for each of 400k edges:
    edge_emb = concat(node_emb[src], node_emb[dst])        # [2D]
    logit    = W2.T @ relu(W1.T @ edge_emb + b1) + b2      # scalar
    eps      = (2*BIAS-1)*eps_u + (1-BIAS)
    aug_w    = sigmoid(log(eps) - log1p(-eps) + logit)
    new_vals = vals * aug_w
Returns (concat([new_vals, new_vals]), mean(aug_w)).

Sharding: edges data-parallel across 8 NeuronCores; node_emb + MLP weights
replicated. The scalar mean is reduced on host from per-core partials.

Device pipeline per core (50176 padded edges = 128 partitions x 392):
  - indirect-DMA gather of node_emb rows (512B each) for src/dst, 49-tile
    chunks, double buffered
  - per 128-edge tile: PE transpose of the gathered [e,d] tile to [d,e],
    DVE copy PSUM->SBUF, two accumulating fp32 matmuls against W1 halves
    giving H.T = [128e, 64h] in PSUM, ACT relu, DVE fused mul+reduce with
    replicated W2 -> per-edge logits
  - one bulk elementwise gate phase (ACT Ln/Sigmoid + DVE) over [128, 392]
"""

import os

import numpy as np

import concourse.bass as bass
import concourse.mybir as mybir
import concourse.tile as tile
from concourse.bass import IndirectOffsetOnAxis
from concourse.bass_utils import run_bass_kernel_spmd

# problem constants
N_NODES = 100000
D = 128
EH = 400000
H = 64
BIAS = 1e-4
N_CORES = 8

P = 128

# fp32 constants matching the reference's arithmetic
C0 = np.float32(BIAS - (1.0 - BIAS))          # -0.9998
C1 = np.float32(1.0 - BIAS)                   # 0.9999
C2 = np.float32(-C0)                          # 0.9998  (scale for 1-eps)
C3 = np.float32(1.0) - C1                     # 1 - fl(0.9999), exact f32

f32 = mybir.dt.float32
i32 = mybir.dt.int32

# results of the last device run (for test harness introspection)
LAST_RUN = {}


def split_waits(nc, maxw=1):
    """Walrus codegen rejects >1 sem wait on one instruction; hoist excess
    waits onto chained wait-only drains inserted just before it."""
    n_split = 0
    for f in nc.m.functions:
        for bb in f.blocks:
            idx = 0
            while idx < len(bb.instructions):
                inst = bb.instructions[idx]
                si = inst.sync_info
                if si is not None and si.on_wait is not None and len(si.on_wait) > maxw:
                    waits = list(si.on_wait)
                    keep = waits[-maxw:]
                    extra = waits[:-maxw]
                    pos = idx
                    for k in range(0, len(extra), maxw):
                        chunk = extra[k : k + maxw]
                        d = mybir.InstEventSemaphore(
                            name=f"{inst.name}_w{k}",
                            ins=[],
                            outs=[],
                        )
                        d.engine = inst.engine
                        d.sync_info = mybir.SyncInfo(on_wait=chunk, on_update=[])
                        nc.register_instruction(d, overwrite=True)
                        bb.instructions.insert(pos, d)
                        pos += 1
                        idx += 1
                    inst.sync_info = mybir.SyncInfo(
                        on_wait=keep, on_update=list(si.on_update or [])
                    )
                    n_split += 1
                idx += 1
    return n_split


def build_program(n_nodes, ept, t_ch, b1_nonzero):
    """Build the per-core SPMD Bass program.

    ept: edge columns per partition (edges per core = 128*ept)
    t_ch: 128-edge tiles per gather chunk (must divide ept)
    """
    assert ept % t_ch == 0
    n_chunk = ept // t_ch

    nc = bass.Bass()

    emb = nc.declare_dram_parameter("emb", [n_nodes, D], f32, isOutput=False)
    idx = nc.declare_dram_parameter("idx", [P, 2 * ept], i32, isOutput=False)
    vals = nc.declare_dram_parameter("vals", [P, ept], f32, isOutput=False)
    eps = nc.declare_dram_parameter("eps", [P, ept], f32, isOutput=False)
    mask = nc.declare_dram_parameter("mask", [P, ept], f32, isOutput=False)
    w1 = nc.declare_dram_parameter("w1", [2 * D, H], f32, isOutput=False)
    w2r = nc.declare_dram_parameter("w2r", [P, H], f32, isOutput=False)
    b1r = nc.declare_dram_parameter("b1r", [P, H], f32, isOutput=False)
    b2r = nc.declare_dram_parameter("b2r", [P, 1], f32, isOutput=False)
    # bias columns for ACT ops: [C1, C3, 0.0]
    cvec = nc.declare_dram_parameter("cvec", [P, 3], f32, isOutput=False)
    idn = nc.declare_dram_parameter("idn", [P, P], f32, isOutput=False)
    out_nv = nc.declare_dram_parameter("out_nv", [P, ept], f32, isOutput=True)
    out_sum = nc.declare_dram_parameter("out_sum", [P, 1], f32, isOutput=True)

    Relu = mybir.ActivationFunctionType.Relu
    Ln = mybir.ActivationFunctionType.Ln
    Sigmoid = mybir.ActivationFunctionType.Sigmoid

    with tile.TileContext(nc) as tc:
        with (
            tc.tile_pool(name="consts", bufs=1) as cpool,
            tc.tile_pool(name="gather", bufs=8) as gpool,
            tc.tile_pool(name="xsb", bufs=4) as xpool,
            tc.tile_pool(name="hsb", bufs=4) as hpool,
            tc.tile_pool(name="acc", bufs=1) as apool,
            tc.tile_pool(name="ps_t", bufs=4, space="PSUM") as pspool,
            tc.tile_pool(name="ps_h", bufs=4, space="PSUM") as pshpool,
        ):
            # ---- load constants / per-core arrays into SBUF ----
            idx_sb = cpool.tile([P, 2 * ept], i32)
            nc.sync.dma_start(out=idx_sb[:], in_=idx[:])
            vals_sb = cpool.tile([P, ept], f32)
            nc.sync.dma_start(out=vals_sb[:], in_=vals[:])
            eps_sb = cpool.tile([P, ept], f32)
            nc.sync.dma_start(out=eps_sb[:], in_=eps[:])
            mask_sb = cpool.tile([P, ept], f32)
            nc.sync.dma_start(out=mask_sb[:], in_=mask[:])
            w1s_sb = cpool.tile([P, H], f32)
            nc.sync.dma_start(out=w1s_sb[:], in_=w1[0:D, :])
            w1d_sb = cpool.tile([P, H], f32)
            nc.sync.dma_start(out=w1d_sb[:], in_=w1[D : 2 * D, :])
            w2r_sb = cpool.tile([P, H], f32)
            nc.sync.dma_start(out=w2r_sb[:], in_=w2r[:])
            b2r_sb = cpool.tile([P, 1], f32)
            nc.sync.dma_start(out=b2r_sb[:], in_=b2r[:])
            cvec_sb = cpool.tile([P, 3], f32)
            nc.sync.dma_start(out=cvec_sb[:], in_=cvec[:])
            idn_sb = cpool.tile([P, P], f32)
            nc.sync.dma_start(out=idn_sb[:], in_=idn[:])
            if b1_nonzero:
                b1r_sb = cpool.tile([P, H], f32)
                nc.sync.dma_start(out=b1r_sb[:], in_=b1r[:])

            logits = apool.tile([P, ept], f32)

            # ---- main loop: per-tile gather (128 rows per indirect DMA,
            # one index per partition: the only HW-supported form) + MLP ----
            for e_col in range(ept):
                gs = gpool.tile([P, D], f32, tag="gs")
                nc.gpsimd.indirect_dma_start(
                    out=gs[:],
                    out_offset=None,
                    in_=emb[:],
                    in_offset=IndirectOffsetOnAxis(
                        ap=idx_sb[:, e_col : e_col + 1], axis=0
                    ),
                )
                gd = gpool.tile([P, D], f32, tag="gd")
                nc.gpsimd.indirect_dma_start(
                    out=gd[:],
                    out_offset=None,
                    in_=emb[:],
                    in_offset=IndirectOffsetOnAxis(
                        ap=idx_sb[:, ept + e_col : ept + e_col + 1], axis=0
                    ),
                )
                # transpose gathered [128e, 128d] -> [128d, 128e]
                ps_s = pspool.tile([P, P], f32, tag="pt")
                nc.tensor.transpose(out=ps_s[:], in_=gs[:], identity=idn_sb[:])
                xs = xpool.tile([P, P], f32, tag="x")
                nc.vector.tensor_copy(out=xs[:], in_=ps_s[:])
                ps_d = pspool.tile([P, P], f32, tag="pt")
                nc.tensor.transpose(out=ps_d[:], in_=gd[:], identity=idn_sb[:])
                xd = xpool.tile([P, P], f32, tag="x")
                nc.vector.tensor_copy(out=xd[:], in_=ps_d[:])

                # H.T = [128e, 64h] = Xs.T@W1s + Xd.T@W1d
                hp = pshpool.tile([P, H], f32, tag="h")
                nc.tensor.matmul(
                    out=hp[:], lhsT=xs[:], rhs=w1s_sb[:], start=True, stop=False
                )
                nc.tensor.matmul(
                    out=hp[:], lhsT=xd[:], rhs=w1d_sb[:], start=False, stop=True
                )

                hs = hpool.tile([P, H], f32, tag="hs")
                if b1_nonzero:
                    nc.vector.tensor_add(out=hs[:], in0=hp[:], in1=b1r_sb[:])
                    nc.scalar.activation(
                        out=hs[:], in_=hs[:], func=Relu, bias=cvec_sb[:, 2:3]
                    )
                else:
                    nc.scalar.activation(
                        out=hs[:], in_=hp[:], func=Relu, bias=cvec_sb[:, 2:3]
                    )

                # logit[e] = sum_h hs[e,h] * W2[h]   (b2 folded into the
                # sigmoid bias in the gate phase)
                scr = hpool.tile([P, H], f32, tag="scr")
                nc.vector.tensor_mul(out=scr[:], in0=hs[:], in1=w2r_sb[:])
                nc.vector.tensor_reduce(
                    out=logits[:, e_col : e_col + 1],
                    in_=scr[:],
                    axis=mybir.AxisListType.X,
                    op=mybir.AluOpType.add,
                )

            # ---- bulk gate phase over [128, ept] ----
            lg1 = apool.tile([P, ept], f32)
            nc.scalar.activation(
                out=lg1[:], in_=eps_sb[:], func=Ln, scale=float(C0), bias=cvec_sb[:, 0:1]
            )
            lg2 = apool.tile([P, ept], f32)
            nc.scalar.activation(
                out=lg2[:], in_=eps_sb[:], func=Ln, scale=float(C2), bias=cvec_sb[:, 1:2]
            )
            gin = apool.tile([P, ept], f32)
            nc.vector.tensor_sub(out=gin[:], in0=lg1[:], in1=lg2[:])
            gin2 = apool.tile([P, ept], f32)
            nc.vector.tensor_add(out=gin2[:], in0=gin[:], in1=logits[:])
            aug = apool.tile([P, ept], f32)
            nc.scalar.activation(out=aug[:], in_=gin2[:], func=Sigmoid, bias=b2r_sb[:, 0:1])
            nv = apool.tile([P, ept], f32)
            nc.vector.tensor_mul(out=nv[:], in0=vals_sb[:], in1=aug[:])
            am = apool.tile([P, ept], f32)
            nc.vector.tensor_mul(out=am[:], in0=aug[:], in1=mask_sb[:])
            s_sb = apool.tile([P, 1], f32)
            nc.vector.tensor_reduce(
                out=s_sb[:], in_=am[:], axis=mybir.AxisListType.X,
                op=mybir.AluOpType.add,
            )

            nc.sync.dma_start(out=out_nv[:], in_=nv[:])
            nc.sync.dma_start(out=out_sum[:], in_=s_sb[:])

    split_waits(nc)
    return nc


_PROGRAM_CACHE = {}


def _get_program(n_nodes, ept, t_ch, b1_nonzero):
    key = (n_nodes, ept, t_ch, b1_nonzero)
    if key not in _PROGRAM_CACHE:
        _PROGRAM_CACHE[key] = build_program(n_nodes, ept, t_ch, b1_nonzero)
    return _PROGRAM_CACHE[key]


def make_in_maps(node_emb, src, dst, vals, eps_u, W1, b1, W2, b2, n_cores, ept):
    """Shard + pad host inputs into per-core input maps."""
    e_core = src.shape[0] // n_cores
    e_pad = P * ept
    assert e_pad >= e_core

    node_emb = np.ascontiguousarray(np.asarray(node_emb, dtype=np.float32))
    src = np.asarray(src).astype(np.int32)
    dst = np.asarray(dst).astype(np.int32)
    vals = np.asarray(vals, dtype=np.float32).reshape(-1)
    eps = np.asarray(eps_u, dtype=np.float32).reshape(-1)
    W1 = np.ascontiguousarray(np.asarray(W1, dtype=np.float32))
    b1 = np.asarray(b1, dtype=np.float32).reshape(-1)
    W2 = np.asarray(W2, dtype=np.float32).reshape(-1)
    b2 = np.asarray(b2, dtype=np.float32).reshape(-1)

    w2r = np.ascontiguousarray(np.tile(W2[None, :], (P, 1)))
    b1r = np.ascontiguousarray(np.tile(b1[None, :], (P, 1)))
    b2r = np.full((P, 1), b2[0], dtype=np.float32)
    cvec = np.tile(np.array([[C1, C3, 0.0]], dtype=np.float32), (P, 1))
    idn = np.eye(P, dtype=np.float32)
    mask = (np.arange(e_pad) < e_core).astype(np.float32).reshape(P, ept)

    def pad_to(x, fill):
        out = np.full(e_pad, fill, dtype=x.dtype)
        out[:e_core] = x
        return out.reshape(P, ept)

    in_maps = []
    for c in range(n_cores):
        sl = slice(c * e_core, (c + 1) * e_core)
        idx_cat = np.concatenate(
            [pad_to(src[sl], 0), pad_to(dst[sl], 0)], axis=1
        ).astype(np.int32)
        in_maps.append(
            {
                "emb": node_emb,
                "idx": np.ascontiguousarray(idx_cat),
                "vals": np.ascontiguousarray(pad_to(vals[sl], 0.0)),
                "eps": np.ascontiguousarray(pad_to(eps[sl], 0.5)),
                "mask": mask,
                "w1": W1,
                "w2r": w2r,
                "b1r": b1r,
                "b2r": b2r,
                "cvec": cvec,
                "idn": idn,
            }
        )
    return in_maps


def postprocess(results, n_cores, e_core):
    """Assemble full outputs from per-core results."""
    new_vals = np.concatenate(
        [np.asarray(results[c]["out_nv"]).reshape(-1)[:e_core] for c in range(n_cores)]
    )
    total = float(
        np.sum([np.asarray(results[c]["out_sum"], dtype=np.float64).sum()
                for c in range(n_cores)])
    )
    mean = np.float32(total / EH)
    sym_vals = np.concatenate([new_vals, new_vals])
    return sym_vals, mean


def _ensure_ntff_hook():
    """The agent image's antenv lacks axon_hooks; inject it and register the
    ctypes NTFF profile hook so run_bass_kernel_spmd(trace=True) works."""
    import sys
    import types

    try:
        import antenv.axon_hooks  # noqa: F401
        return
    except ImportError:
        pass
    m = types.ModuleType("antenv.axon_hooks")
    state = {"hook": None}
    m.get_axon_ntff_profile_hook = lambda: state["hook"]
    m.set_axon_ntff_profile_hook = lambda h: state.update(hook=h)
    sys.modules["antenv.axon_hooks"] = m
    try:
        from trn_agent_boot.trn_boot import _ntff_profile_via_ctypes

        state["hook"] = _ntff_profile_via_ctypes("/opt/axon/libaxon_pjrt.so")
    except Exception:
        pass


def kernel(node_emb, src, dst, vals, eps_u, W1, b1, W2, b2):
    ept, t_ch = 392, 49  # 128*392 = 50176 padded edges/core, 8 gather chunks
    b1_nonzero = bool(np.any(np.asarray(b1)))
    nc = _get_program(N_NODES, ept, t_ch, b1_nonzero)
    in_maps = make_in_maps(
        node_emb, src, dst, vals, eps_u, W1, b1, W2, b2, N_CORES, ept
    )
    trace = bool(int(os.environ.get("BASS_KERNEL_TRACE", "0")))
    if trace:
        _ensure_ntff_hook()
    res = run_bass_kernel_spmd(nc, in_maps, list(range(N_CORES)), trace=trace)
    LAST_RUN["exec_time_ns"] = res.exec_time_ns
    LAST_RUN["profile_json"] = getattr(res, "profile_json", None)
    return postprocess(res.results, N_CORES, EH // N_CORES)
